# revision 9
# baseline (speedup 1.0000x reference)
"""Trainium2 Bass kernel for DifferentiableRasterizer (point-to-mesh distance field).

out[b, n] = exp(-100 * min_f dist^2(points[b,n], tri[b,f]))

Strategy (8-core data-parallel, points axis sharded; tri_verts replicated):
  Host precomputes, per (batch, face), an orthonormal per-segment frame so the
  point-triangle distance decomposes into squares of AFFINE functionals of p:
     dist^2(p, seg_i) = ip_i(p)^2 + delta_i^2,  delta = max(sig-l, min(sig, 0))
     plane^2          = dnp(p)^2
     inside          <=> max_i ip_i(p) <= 0   (ip oriented outward)
     dist^2(p, tri)   = dnp^2 + (inside ? 0 : min_i dist2d_i)
  The 7 affine functionals per face (sig01,sig02,sig12, ip01,ip02,ip12, dnp)
  are computed on the TensorEngine as K=4 matmuls (homogeneous points), and the
  nonlinear tail runs on ACT/DVE/GPSIMD in a face-major layout
  (128 faces on partitions, points along the free dim).
"""

import numpy as np

B = 4
N = 8192
F = 1024
NCORES = 8
NP = N // NCORES          # points per core (per batch)
PC = 512                  # point-chunk (free dim)
NPC = NP // PC            # point chunks per batch
NFC = F // 128            # face chunks per batch
ALPHA = 100.0
MIN_TRI_AREA = 1e-5
BIGVAL = 1e18

# PE matmul dtype mode:
#   "fp32"   exact, 4 cyc/row
#   "fp32r"  1 cyc/row @ N>=256, ~tf32 precision (~5e-3 out err)
#   "bf16x4" 1 cyc/row, K=16 two-limb bf16 split per operand (~1e-4 out err)
MM_MODE = "bf16x4"
KDIM = 16 if MM_MODE == "bf16x4" else 4
USE_CUSTOM = True  # fused custom-DVE ops (7 DVE passes/chunk) vs stock ops
TRACE = False  # set True (before first kernel() call) to capture an NTFF profile

BIGFILL = 1e30   # "not a candidate" fill for inside-masked segment distances
BIGTH = 1e29     # threshold detecting the fill


_DVE_OPS = {}


def _register_custom_ops():
    """Register the two fused DVE ops (idempotent)."""
    if _DVE_OPS:
        return _DVE_OPS
    from concourse.dve_spec import (
        Spec, Src0, Src1, C0, C2, Zero, lower, maxx, minn, select, sq,
    )
    from concourse.dve_ops import DveOp, OPS, get_dve_sub_opcode, has_src1
    from concourse.dve_uop import DveOpSpec
    import numpy as _np

    import concourse.dve_ops as dve_ops_mod

    def _mk(name, spec):
        for op in OPS:
            if op.name == name:
                _DVE_OPS[name] = op
                return
        shas = {}
        op = DveOp(name, spec, subdim=False, uops_sha=shas)
        OPS.append(op)
        # the module builds these maps at import; extend them for new ops
        dve_ops_mod._SUB_OPCODE_FOR_NAME[name] = (
            dve_ops_mod._CUSTOM_DVE_ROW_BASE + len(OPS) - 1
        )
        dve_ops_mod.CUSTOM_DVE_SPECS[name] = spec
        for ver in ("v3", "v4"):
            s = DveOpSpec(
                name=name,
                opcode=get_dve_sub_opcode(name),
                uops=lower(spec, ver=ver),
                rd1_en=has_src1(spec),
            )
            shas[ver] = s.sha(ver)
        _DVE_OPS[name] = op

    # B = select(ip > 0, ip^2 + max(sig + s0, min(sig, 0))^2, BIGFILL)
    #   in0 = sig, in1 = ip, s0 = NEGATED segment length (per-partition),
    #   imm2 = BIGFILL
    _mk(
        "RAST_SEGSEL_ANT",
        Spec(
            body=select(
                Src1 > Zero,
                sq(Src1) + sq(maxx(Src0 + C0, minn(Src0, Zero))),
                C2,
            ),
            reference=lambda in0, in1, s0, imm2: _np.where(
                in1 > 0,
                in1 * in1
                + _np.square(_np.maximum(in0 + s0, _np.minimum(in0, 0.0))),
                imm2,
            ),
        ),
    )
    # fin = dnp^2 + (M >= BIGTH ? 0 : M);  in0 = M, in1 = dnp, s0 = BIGTH
    _mk(
        "RAST_FINPL_ANT",
        Spec(
            body=sq(Src1) + select(Src0 >= C0, Zero, Src0),
            reference=lambda in0, in1, s0: in1 * in1
            + _np.where(in0 >= s0, 0.0, in0),
        ),
    )
    return _DVE_OPS


def _host_face_constants(tri):
    """tri: (B, F, 3, 3) float32 -> per-face affine functional rows (float64)."""
    t = tri.astype(np.float64)
    v0, v1, v2 = t[:, :, 0, :], t[:, :, 1, :], t[:, :, 2, :]
    e0 = v1 - v0
    e1 = v2 - v0
    e12 = v2 - v1
    n = np.cross(e0, e1)
    area2 = (n * n).sum(-1)
    valid = area2 >= 4.0 * (MIN_TRI_AREA ** 2)
    nh = n / np.sqrt(np.maximum(area2, 1e-300))[..., None]

    def seg_const(a, d, opp):
        L = np.sqrt((d * d).sum(-1))
        eh = d / np.maximum(L, 1e-300)[..., None]
        m = np.cross(eh, nh)
        flip = (m * (opp - a)).sum(-1) > 0
        m = np.where(flip[..., None], -m, m)
        # sigma(p) = eh.p + eo ; ip(p) = m.p + mo
        return eh, -(eh * a).sum(-1), m, -(m * a).sum(-1), L

    segs = [seg_const(v0, e0, v2), seg_const(v0, e1, v1), seg_const(v1, e12, v0)]
    dn_c, dn_o = nh, -(nh * v0).sum(-1)

    inv = ~valid
    fixed = []
    for eh, eo, m, mo, L in segs:
        eh = np.where(inv[..., None], 0.0, eh)
        eo = np.where(inv, 0.0, eo)
        m = np.where(inv[..., None], 0.0, m)
        mo = np.where(inv, BIGVAL, mo)
        L = np.where(inv, 1.0, L)
        fixed.append((eh, eo, m, mo, L))
    dn_c = np.where(inv[..., None], 0.0, dn_c)
    dn_o = np.where(inv, BIGVAL, dn_o)
    return fixed, dn_c, dn_o, valid


def _host_pack(points, tri):
    """Build the DRAM input arrays for the device kernel."""
    segs, dn_c, dn_o, valid = _host_face_constants(tri)

    # G matrix: [B, NFC, 7, 4, 128]  (functional rows over homogeneous p)
    # functional order: sig01, sig02, sig12, ip01, ip02, ip12, dnp
    G = np.zeros((B, NFC, 7, 4, 128), np.float32)
    for k in range(3):
        eh, eo, m, mo, _ = segs[k]
        for b in range(B):
            gc = eh[b].reshape(NFC, 128, 3)
            go = eo[b].reshape(NFC, 128)
            G[b, :, k, 0:3, :] = gc.transpose(0, 2, 1)
            G[b, :, k, 3, :] = go
            ic = m[b].reshape(NFC, 128, 3)
            io = mo[b].reshape(NFC, 128)
            G[b, :, 3 + k, 0:3, :] = ic.transpose(0, 2, 1)
            G[b, :, 3 + k, 3, :] = io
    for b in range(B):
        nc_ = dn_c[b].reshape(NFC, 128, 3)
        no_ = dn_o[b].reshape(NFC, 128)
        G[b, :, 6, 0:3, :] = nc_.transpose(0, 2, 1)
        G[b, :, 6, 3, :] = no_

    # negated segment lengths for ACT bias: [128, B*NFC*3]
    lneg = np.zeros((128, B * NFC * 3), np.float32)
    for b in range(B):
        for k in range(3):
            L = segs[k][4][b].reshape(NFC, 128)
            for fc in range(NFC):
                lneg[:, (b * NFC + fc) * 3 + k] = -L[fc]

    eye = np.eye(128, dtype=np.float16)

    # homogeneous point tiles per core: [B, 4, NP] fp32
    pts_full = []
    for c in range(NCORES):
        ps = points[:, c * NP:(c + 1) * NP, :].astype(np.float32)  # (B, NP, 3)
        pt = np.ones((B, 4, NP), np.float32)
        pt[:, 0:3, :] = ps.transpose(0, 2, 1)
        pts_full.append(pt)

    if MM_MODE == "bf16x4":
        import ml_dtypes

        bf16 = ml_dtypes.bfloat16
        # two-limb bf16 split: x = hi + lo (+O(2^-18))
        Ghi = G.astype(bf16)
        Glo = (G - Ghi.astype(np.float32)).astype(bf16)
        # lhsT rows (K=16): [Ghi; Glo; Ghi; Glo]
        Gk = np.concatenate([Ghi, Glo, Ghi, Glo], axis=3)  # [B,NFC,7,16,128]
        Gk = np.ascontiguousarray(Gk.transpose(0, 3, 1, 2, 4)).reshape(
            B, KDIM, NFC * 7 * 128
        )
        ptils = []
        for pt in pts_full:
            phi = pt.astype(bf16)
            plo = (pt - phi.astype(np.float32)).astype(bf16)
            # rhs rows (K=16): [phi; phi; plo; plo]
            ptils.append(np.concatenate([phi, phi, plo, plo], axis=1))
        return Gk, lneg, eye, ptils, valid

    # fp32/fp32r: K=4, pre-transpose for direct DMA (K on partitions)
    Gk = np.ascontiguousarray(G.transpose(0, 3, 1, 2, 4)).reshape(
        B, 4, NFC * 7 * 128
    )
    return Gk, lneg, eye, pts_full, valid


def _host_invalid_min(points, tri, valid):
    """Exact min dist^2 over INVALID faces only (numpy, usually none)."""
    if valid.all():
        return None
    out = np.full((B, N), np.inf, np.float64)
    for b in range(B):
        idx = np.where(~valid[b])[0]
        if len(idx) == 0:
            continue
        t = tri[b, idx].astype(np.float64)   # (Fi, 3, 3)
        p = points[b].astype(np.float64)     # (N, 3)
        v0, v1, v2 = t[:, 0], t[:, 1], t[:, 2]

        def segd(a, d):
            L2 = np.maximum((d * d).sum(-1), 1e-12)
            tt = np.clip(((p[:, None, :] - a) * d).sum(-1) / L2, 0, 1)
            proj = a + tt[..., None] * d
            df = p[:, None, :] - proj
            return (df * df).sum(-1)

        dd = np.minimum(np.minimum(segd(v0, v1 - v0), segd(v0, v2 - v0)),
                        segd(v1, v2 - v1))
        out[b] = dd.min(-1)
    return out


def _build_bass(reps=1):
    import concourse.bass as bass
    import concourse.bacc as bacc
    import concourse.tile as tile
    from concourse import mybir

    f32 = mybir.dt.float32
    nc = bacc.Bacc(None)

    mmdt_in = {
        "fp32": f32,
        "fp32r": mybir.dt.float32r,
        "bf16x4": mybir.dt.bfloat16,
    }[MM_MODE]
    ptil = nc.declare_dram_parameter("ptil", [B, KDIM, NP], mmdt_in, isOutput=False)
    gmat = nc.declare_dram_parameter("gmat", [B, KDIM, NFC * 7 * 128], mmdt_in, isOutput=False)
    lneg = nc.declare_dram_parameter("lneg", [128, B * NFC * 3], f32, isOutput=False)
    eye = nc.declare_dram_parameter("eye", [128, 128], mybir.dt.float16, isOutput=False)
    outp = nc.declare_dram_parameter("out", [B, NP], f32, isOutput=True)

    mm_dt = f32 if MM_MODE == "fp32" else mybir.dt.float32r

    ACT = mybir.ActivationFunctionType
    ALU = mybir.AluOpType
    f16 = mybir.dt.float16

    # matmul free-dim cap is 512 fp32 (one PSUM bank); the wide [128, NP]
    # PSUM tiles are filled by MMH=NP//512 paired matmuls and consumed by
    # single wide DVE/ACT ops (halves the per-op overhead + instruction count)
    MMH = NP // 512

    with tile.TileContext(nc) as tc:
        with (
            tc.tile_pool(name="const", bufs=1) as constp,
            tc.tile_pool(name="gp", bufs=2) as gpool,
            tc.tile_pool(name="pp", bufs=2) as ppool,
            tc.tile_pool(name="ps", bufs=1, space="PSUM") as psum,
            tc.tile_pool(name="pst", bufs=2, space="PSUM") as psum_t,
            tc.tile_pool(name="wk", bufs=2) as wk,
            tc.tile_pool(name="accp", bufs=2) as accp,
            tc.tile_pool(name="outs", bufs=2) as outsp,
        ):
            ltile = constp.tile([128, B * NFC * 3], f32, tag="lneg")
            nc.sync.dma_start(ltile[:], lneg[:])
            eyet = constp.tile([128, 128], f16, tag="eye")
            nc.sync.dma_start(eyet[:], eye[:])

            ops = _register_custom_ops()
            rot = [0]  # global PSUM tag rotation across fc/b

            def functional(gt, pt, fc, phi):
                pst = psum.tile([128, NP], f32, tag=f"ps_{rot[0] % 3}")
                rot[0] += 1
                lhsT = gt[:, (fc * 7 + phi) * 128:(fc * 7 + phi + 1) * 128]
                for h in range(MMH):
                    nc.tensor.matmul(
                        pst[:, h * 512:(h + 1) * 512], lhsT,
                        pt[:, h * 512:(h + 1) * 512], start=True, stop=True,
                    )
                return pst

            for rep in range(reps):
              for b in range(B):
                gt = gpool.tile([KDIM, NFC * 7 * 128], mmdt_in, tag="g")
                nc.sync.dma_start(gt[:], gmat[b])
                pt = ppool.tile([KDIM, NP], mmdt_in, tag="p")
                nc.sync.dma_start(pt[:], ptil[b])

                acc = accp.tile([128, NP], f16, tag="acc")
                nc.vector.memset(acc[:], 60000.0)

                for fc in range(NFC):
                    # interleave MMs with consumers so only ~2 PSUM tiles
                    # are live and the DVE min-chain starts early
                    ics, Bt = [], []

                    def seg(k):
                        lb = ltile[:, (b * NFC + fc) * 3 + k:
                                   (b * NFC + fc) * 3 + k + 1]
                        Bk = wk.tile([128, NP], f16, tag=f"B_{k}")
                        nc.vector._custom_dve(
                            ops["RAST_SEGSEL_ANT"],
                            out=Bk[:], in0=sig[:], in1=ics[k][:],
                            s0=lb, imm2=BIGFILL,
                        )
                        Bt.append(Bk)

                    for k, (iphi, sphi) in enumerate(((3, 0), (4, 1), (5, 2))):
                        ipf = functional(gt, pt, fc, iphi)
                        ic = wk.tile([128, NP], f32, tag=f"ic_{k}")
                        nc.scalar.activation(ic[:], ipf[:], ACT.Copy)
                        ics.append(ic)
                        sig = functional(gt, pt, fc, sphi)
                        seg(k)
                        if k == 1:
                            m1 = wk.tile([128, NP], f16, tag="m1")
                            nc.vector.tensor_tensor(
                                m1[:], Bt[0][:], Bt[1][:], op=ALU.min)
                    dnp = functional(gt, pt, fc, 6)
                    m2 = wk.tile([128, NP], f16, tag="m2")
                    nc.vector.tensor_tensor(m2[:], m1[:], Bt[2][:], op=ALU.min)
                    fin = wk.tile([128, NP], f16, tag="fin")
                    nc.vector._custom_dve(
                        ops["RAST_FINPL_ANT"],
                        out=fin[:], in0=m2[:], in1=dnp[:], s0=BIGTH,
                    )
                    nc.vector.tensor_tensor(acc[:], acc[:], fin[:], op=ALU.min)

                # --- tail: min over the 128 face-slots (partitions) ---
                nj = NP // 128
                dmin = outsp.tile([128, nj], f32, tag="dmin")
                for j in range(nj):
                    tp = psum_t.tile([128, 128], f16, tag="tp")
                    nc.tensor.transpose(tp[:], acc[:, j * 128:(j + 1) * 128], eyet[:])
                    nc.vector.tensor_reduce(
                        dmin[:, j:j + 1], tp[:], axis=mybir.AxisListType.X, op=ALU.min
                    )
                eo = outsp.tile([128, nj], f32, tag="eo")
                nc.scalar.activation(eo[:], dmin[:], ACT.Exp, scale=-ALPHA)
                dst = outp[b].rearrange("(j p) -> p j", p=128)
                nc.sync.dma_start(dst, eo[:])

    nc.finalize()
    return nc


_CACHED = {}


def kernel(points: np.ndarray, tri_verts: np.ndarray) -> np.ndarray:
    points = np.asarray(points)
    tri_verts = np.asarray(tri_verts)
    assert points.shape == (B, N, 3) and tri_verts.shape == (B, F, 3, 3)

    from concourse.bass_utils import run_bass_kernel_spmd

    G, lneg, eye, ptils, valid = _host_pack(points, tri_verts)

    if "nc" not in _CACHED:
        _CACHED["nc"] = _build_bass()
    nc = _CACHED["nc"]

    in_maps = [
        {"ptil": ptils[c], "gmat": G, "lneg": lneg, "eye": eye}
        for c in range(NCORES)
    ]
    res = run_bass_kernel_spmd(
        nc, in_maps, core_ids=list(range(NCORES)), trace=TRACE
    )
    _CACHED["last_exec_ns"] = res.exec_time_ns
    _CACHED["last_res"] = res

    out = np.zeros((B, N), np.float32)
    for c in range(NCORES):
        out[:, c * NP:(c + 1) * NP] = res.results[c]["out"]

    inv_min = _host_invalid_min(points, tri_verts, valid)
    if inv_min is not None:
        out = np.maximum(out, np.exp(-ALPHA * inv_min).astype(np.float32))
    return out



# revision 11
# speedup vs baseline: 1.3353x; 1.3353x over previous
"""Trainium2 Bass kernel for DifferentiableRasterizer (point-to-mesh distance field).

out[b, n] = exp(-100 * min_f dist^2(points[b,n], tri[b,f]))

Strategy (8-core data-parallel, points axis sharded; tri_verts replicated):
  Host precomputes, per (batch, face), an orthonormal per-segment frame so the
  point-triangle distance decomposes into squares of AFFINE functionals of p:
     dist^2(p, seg_i) = ip_i(p)^2 + delta_i^2,  delta = max(sig-l, min(sig, 0))
     plane^2          = dnp(p)^2
     inside          <=> max_i ip_i(p) <= 0   (ip oriented outward)
     dist^2(p, tri)   = dnp^2 + (inside ? 0 : min_i dist2d_i)
  The 7 affine functionals per face (sig01,sig02,sig12, ip01,ip02,ip12, dnp)
  are computed on the TensorEngine as K=4 matmuls (homogeneous points), and the
  nonlinear tail runs on ACT/DVE/GPSIMD in a face-major layout
  (128 faces on partitions, points along the free dim).
"""

import numpy as np

B = 4
N = 8192
F = 1024
NCORES = 8
NP = N // NCORES          # points per core (per batch)
PC = 512                  # point-chunk (free dim)
NPC = NP // PC            # point chunks per batch
NFC = F // 128            # face chunks per batch
ALPHA = 100.0
MIN_TRI_AREA = 1e-5
BIGVAL = 1e18

# PE matmul dtype mode:
#   "fp32"   exact, 4 cyc/row
#   "fp32r"  1 cyc/row @ N>=256, ~tf32 precision (~5e-3 out err)
#   "bf16x4" 1 cyc/row, K=16 two-limb bf16 split per operand (~1e-4 out err)
MM_MODE = "bf16x4"
KDIM = 16 if MM_MODE == "bf16x4" else 4
USE_CUSTOM = True  # fused custom-DVE ops (7 DVE passes/chunk) vs stock ops
TRACE = False  # set True (before first kernel() call) to capture an NTFF profile

BIGFILL = 1e30   # "not a candidate" fill for inside-masked segment distances
BIGTH = 1e29     # threshold detecting the fill


_DVE_OPS = {}


def _register_custom_ops():
    """Register the two fused DVE ops (idempotent)."""
    if _DVE_OPS:
        return _DVE_OPS
    from concourse.dve_spec import (
        Spec, Src0, Src1, C0, C2, Zero, lower, maxx, minn, select, sq,
    )
    from concourse.dve_ops import DveOp, OPS, get_dve_sub_opcode, has_src1
    from concourse.dve_uop import DveOpSpec
    import numpy as _np

    import concourse.dve_ops as dve_ops_mod

    def _mk(name, spec):
        for op in OPS:
            if op.name == name:
                _DVE_OPS[name] = op
                return
        shas = {}
        op = DveOp(name, spec, subdim=False, uops_sha=shas)
        OPS.append(op)
        # the module builds these maps at import; extend them for new ops
        dve_ops_mod._SUB_OPCODE_FOR_NAME[name] = (
            dve_ops_mod._CUSTOM_DVE_ROW_BASE + len(OPS) - 1
        )
        dve_ops_mod.CUSTOM_DVE_SPECS[name] = spec
        for ver in ("v3", "v4"):
            s = DveOpSpec(
                name=name,
                opcode=get_dve_sub_opcode(name),
                uops=lower(spec, ver=ver),
                rd1_en=has_src1(spec),
            )
            shas[ver] = s.sha(ver)
        _DVE_OPS[name] = op

    # B = select(ip > 0, ip^2 + max(sig + s0, min(sig, 0))^2, BIGFILL)
    #   in0 = sig, in1 = ip, s0 = NEGATED segment length (per-partition),
    #   imm2 = BIGFILL
    _mk(
        "RAST_SEGSEL_ANT",
        Spec(
            body=select(
                Src1 > Zero,
                sq(Src1) + sq(maxx(Src0 + C0, minn(Src0, Zero))),
                C2,
            ),
            reference=lambda in0, in1, s0, imm2: _np.where(
                in1 > 0,
                in1 * in1
                + _np.square(_np.maximum(in0 + s0, _np.minimum(in0, 0.0))),
                imm2,
            ),
        ),
    )
    # fin = dnp^2 + (M >= BIGTH ? 0 : M);  in0 = M, in1 = dnp, s0 = BIGTH
    _mk(
        "RAST_FINPL_ANT",
        Spec(
            body=sq(Src1) + select(Src0 >= C0, Zero, Src0),
            reference=lambda in0, in1, s0: in1 * in1
            + _np.where(in0 >= s0, 0.0, in0),
        ),
    )
    return _DVE_OPS


def _host_face_constants(tri):
    """tri: (B, F, 3, 3) float32 -> per-face affine functional rows (float64)."""
    t = tri.astype(np.float64)
    v0, v1, v2 = t[:, :, 0, :], t[:, :, 1, :], t[:, :, 2, :]
    e0 = v1 - v0
    e1 = v2 - v0
    e12 = v2 - v1
    n = np.cross(e0, e1)
    area2 = (n * n).sum(-1)
    valid = area2 >= 4.0 * (MIN_TRI_AREA ** 2)
    nh = n / np.sqrt(np.maximum(area2, 1e-300))[..., None]

    def seg_const(a, d, opp):
        L = np.sqrt((d * d).sum(-1))
        eh = d / np.maximum(L, 1e-300)[..., None]
        m = np.cross(eh, nh)
        flip = (m * (opp - a)).sum(-1) > 0
        m = np.where(flip[..., None], -m, m)
        # sigma(p) = eh.p + eo ; ip(p) = m.p + mo
        return eh, -(eh * a).sum(-1), m, -(m * a).sum(-1), L

    segs = [seg_const(v0, e0, v2), seg_const(v0, e1, v1), seg_const(v1, e12, v0)]
    dn_c, dn_o = nh, -(nh * v0).sum(-1)

    inv = ~valid
    fixed = []
    for eh, eo, m, mo, L in segs:
        eh = np.where(inv[..., None], 0.0, eh)
        eo = np.where(inv, 0.0, eo)
        m = np.where(inv[..., None], 0.0, m)
        mo = np.where(inv, BIGVAL, mo)
        L = np.where(inv, 1.0, L)
        fixed.append((eh, eo, m, mo, L))
    dn_c = np.where(inv[..., None], 0.0, dn_c)
    dn_o = np.where(inv, BIGVAL, dn_o)
    return fixed, dn_c, dn_o, valid


def _host_pack(points, tri):
    """Build the DRAM input arrays for the device kernel."""
    segs, dn_c, dn_o, valid = _host_face_constants(tri)

    # G matrix: [B, NFC, 7, 4, 128]  (functional rows over homogeneous p)
    # functional order: sig01, sig02, sig12, ip01, ip02, ip12, dnp
    G = np.zeros((B, NFC, 7, 4, 128), np.float32)
    for k in range(3):
        eh, eo, m, mo, _ = segs[k]
        for b in range(B):
            gc = eh[b].reshape(NFC, 128, 3)
            go = eo[b].reshape(NFC, 128)
            G[b, :, k, 0:3, :] = gc.transpose(0, 2, 1)
            G[b, :, k, 3, :] = go
            ic = m[b].reshape(NFC, 128, 3)
            io = mo[b].reshape(NFC, 128)
            G[b, :, 3 + k, 0:3, :] = ic.transpose(0, 2, 1)
            G[b, :, 3 + k, 3, :] = io
    for b in range(B):
        nc_ = dn_c[b].reshape(NFC, 128, 3)
        no_ = dn_o[b].reshape(NFC, 128)
        G[b, :, 6, 0:3, :] = nc_.transpose(0, 2, 1)
        G[b, :, 6, 3, :] = no_

    # negated segment lengths for ACT bias: [128, B*NFC*3]
    lneg = np.zeros((128, B * NFC * 3), np.float32)
    for b in range(B):
        for k in range(3):
            L = segs[k][4][b].reshape(NFC, 128)
            for fc in range(NFC):
                lneg[:, (b * NFC + fc) * 3 + k] = -L[fc]

    eye = np.eye(128, dtype=np.float16)

    # homogeneous point tiles per core: [B, 4, NP] fp32
    pts_full = []
    for c in range(NCORES):
        ps = points[:, c * NP:(c + 1) * NP, :].astype(np.float32)  # (B, NP, 3)
        pt = np.ones((B, 4, NP), np.float32)
        pt[:, 0:3, :] = ps.transpose(0, 2, 1)
        pts_full.append(pt)

    if MM_MODE == "bf16x4":
        import ml_dtypes

        bf16 = ml_dtypes.bfloat16
        # two-limb bf16 split: x = hi + lo (+O(2^-18))
        Ghi = G.astype(bf16)
        Glo = (G - Ghi.astype(np.float32)).astype(bf16)
        # lhsT rows (K=16): [Ghi; Glo; Ghi; Glo]
        Gk = np.concatenate([Ghi, Glo, Ghi, Glo], axis=3)  # [B,NFC,7,16,128]
        Gk = np.ascontiguousarray(Gk.transpose(0, 3, 1, 2, 4)).reshape(
            B, KDIM, NFC * 7 * 128
        )
        ptils = []
        for pt in pts_full:
            phi = pt.astype(bf16)
            plo = (pt - phi.astype(np.float32)).astype(bf16)
            # rhs rows (K=16): [phi; phi; plo; plo]
            ptils.append(np.concatenate([phi, phi, plo, plo], axis=1))
        return Gk, lneg, eye, ptils, valid

    # fp32/fp32r: K=4, pre-transpose for direct DMA (K on partitions)
    Gk = np.ascontiguousarray(G.transpose(0, 3, 1, 2, 4)).reshape(
        B, 4, NFC * 7 * 128
    )
    return Gk, lneg, eye, pts_full, valid


def _host_invalid_min(points, tri, valid):
    """Exact min dist^2 over INVALID faces only (numpy, usually none)."""
    if valid.all():
        return None
    out = np.full((B, N), np.inf, np.float64)
    for b in range(B):
        idx = np.where(~valid[b])[0]
        if len(idx) == 0:
            continue
        t = tri[b, idx].astype(np.float64)   # (Fi, 3, 3)
        p = points[b].astype(np.float64)     # (N, 3)
        v0, v1, v2 = t[:, 0], t[:, 1], t[:, 2]

        def segd(a, d):
            L2 = np.maximum((d * d).sum(-1), 1e-12)
            tt = np.clip(((p[:, None, :] - a) * d).sum(-1) / L2, 0, 1)
            proj = a + tt[..., None] * d
            df = p[:, None, :] - proj
            return (df * df).sum(-1)

        dd = np.minimum(np.minimum(segd(v0, v1 - v0), segd(v0, v2 - v0)),
                        segd(v1, v2 - v1))
        out[b] = dd.min(-1)
    return out


def _build_bass(reps=1):
    import concourse.bass as bass
    import concourse.bacc as bacc
    import concourse.tile as tile
    from concourse import mybir

    f32 = mybir.dt.float32
    nc = bacc.Bacc(None)

    mmdt_in = {
        "fp32": f32,
        "fp32r": mybir.dt.float32r,
        "bf16x4": mybir.dt.bfloat16,
    }[MM_MODE]
    ptil = nc.declare_dram_parameter("ptil", [B, KDIM, NP], mmdt_in, isOutput=False)
    gmat = nc.declare_dram_parameter("gmat", [B, KDIM, NFC * 7 * 128], mmdt_in, isOutput=False)
    lneg = nc.declare_dram_parameter("lneg", [128, B * NFC * 3], f32, isOutput=False)
    eye = nc.declare_dram_parameter("eye", [128, 128], mybir.dt.float16, isOutput=False)
    outp = nc.declare_dram_parameter("out", [B, NP], f32, isOutput=True)

    mm_dt = f32 if MM_MODE == "fp32" else mybir.dt.float32r

    ACT = mybir.ActivationFunctionType
    ALU = mybir.AluOpType
    f16 = mybir.dt.float16

    with tile.TileContext(nc) as tc:
        with (
            tc.tile_pool(name="const", bufs=1) as constp,
            tc.tile_pool(name="gp", bufs=2) as gpool,
            tc.tile_pool(name="pp", bufs=2) as ppool,
            tc.tile_pool(name="ps", bufs=1, space="PSUM") as psum,
            tc.tile_pool(name="pst", bufs=1, space="PSUM") as psum_t,
            tc.tile_pool(name="wk", bufs=2) as wk,
            tc.tile_pool(name="accp", bufs=2) as accp,
            tc.tile_pool(name="outs", bufs=2) as outsp,
        ):
            ltile = constp.tile([128, B * NFC * 3], f32, tag="lneg")
            nc.sync.dma_start(ltile[:], lneg[:])
            eyet = constp.tile([128, 128], f16, tag="eye")
            nc.sync.dma_start(eyet[:], eye[:])

            ops = _register_custom_ops()

            for rep in range(reps):
              for b in range(B):
                gt = gpool.tile([KDIM, NFC * 7 * 128], mmdt_in, tag="g")
                nc.sync.dma_start(gt[:], gmat[b])
                pt = ppool.tile([KDIM, NP], mmdt_in, tag="p")
                nc.sync.dma_start(pt[:], ptil[b])

                for pc in range(NPC):
                    rhs = pt[:, pc * PC:(pc + 1) * PC]
                    acc = accp.tile([128, PC], f16, tag="acc")

                    for fc in range(NFC):
                        # --- PE: 7 affine functionals -> PSUM ---
                        # ip's first (ACT copies drain those banks early),
                        # then sigs, dnp last (consumed last by finpl).
                        fn = {}
                        for phi, name in (
                            (3, "i01"), (4, "i02"), (5, "i12"),
                            (0, "s01"), (1, "s02"), (2, "s12"), (6, "dnp"),
                        ):
                            pst = psum.tile([128, PC], f32, tag=f"ps_{name}")
                            lhsT = gt[:, (fc * 7 + phi) * 128:(fc * 7 + phi + 1) * 128]
                            nc.tensor.matmul(pst[:], lhsT, rhs, start=True, stop=True)
                            fn[name] = pst

                        # ip columns to SBUF via ACT (frees a PSUM read for DVE)
                        ics = []
                        for k, inm in enumerate(("i01", "i02", "i12")):
                            ic = wk.tile([128, PC], f32, tag=f"ic_{k}")
                            nc.scalar.activation(ic[:], fn[inm][:], ACT.Copy)
                            ics.append(ic)
                        Bt = []
                        for k, sn in enumerate(("s01", "s02", "s12")):
                            lb = ltile[:, (b * NFC + fc) * 3 + k:
                                       (b * NFC + fc) * 3 + k + 1]
                            Bk = wk.tile([128, PC], f16, tag=f"B_{k}")
                            nc.vector._custom_dve(
                                ops["RAST_SEGSEL_ANT"],
                                out=Bk[:], in0=fn[sn][:], in1=ics[k][:],
                                s0=lb, imm2=BIGFILL,
                            )
                            Bt.append(Bk)
                            if k == 1:
                                m1 = wk.tile([128, PC], f16, tag="m1")
                                nc.vector.tensor_tensor(
                                    m1[:], Bt[0][:], Bt[1][:], op=ALU.min)
                        m2 = wk.tile([128, PC], f16, tag="m2")
                        nc.vector.tensor_tensor(m2[:], m1[:], Bt[2][:], op=ALU.min)
                        if fc == 0:
                            # first face-chunk: finpl writes acc directly
                            # (no memset / no separate accmin needed)
                            nc.vector._custom_dve(
                                ops["RAST_FINPL_ANT"],
                                out=acc[:], in0=m2[:], in1=fn["dnp"][:], s0=BIGTH,
                            )
                        else:
                            fin = wk.tile([128, PC], f16, tag="fin")
                            nc.vector._custom_dve(
                                ops["RAST_FINPL_ANT"],
                                out=fin[:], in0=m2[:], in1=fn["dnp"][:], s0=BIGTH,
                            )
                            nc.vector.tensor_tensor(acc[:], acc[:], fin[:], op=ALU.min)

                    # --- tail: min over the 128 face-slots (partitions) ---
                    nj = PC // 128
                    dmin = outsp.tile([128, nj], f32, tag="dmin")
                    for j in range(nj):
                        tp = psum_t.tile([128, 128], f16, tag="tp")
                        nc.tensor.transpose(tp[:], acc[:, j * 128:(j + 1) * 128], eyet[:])
                        nc.vector.tensor_reduce(
                            dmin[:, j:j + 1], tp[:], axis=mybir.AxisListType.X, op=ALU.min
                        )
                    eo = outsp.tile([128, nj], f32, tag="eo")
                    nc.scalar.activation(eo[:], dmin[:], ACT.Exp, scale=-ALPHA)
                    dst = outp[b, pc * PC:(pc + 1) * PC].rearrange("(j p) -> p j", p=128)
                    nc.sync.dma_start(dst, eo[:])

    nc.finalize()
    return nc


_CACHED = {}


def kernel(points: np.ndarray, tri_verts: np.ndarray) -> np.ndarray:
    points = np.asarray(points)
    tri_verts = np.asarray(tri_verts)
    assert points.shape == (B, N, 3) and tri_verts.shape == (B, F, 3, 3)

    from concourse.bass_utils import run_bass_kernel_spmd

    G, lneg, eye, ptils, valid = _host_pack(points, tri_verts)

    if "nc" not in _CACHED:
        _CACHED["nc"] = _build_bass()
    nc = _CACHED["nc"]

    in_maps = [
        {"ptil": ptils[c], "gmat": G, "lneg": lneg, "eye": eye}
        for c in range(NCORES)
    ]
    res = run_bass_kernel_spmd(
        nc, in_maps, core_ids=list(range(NCORES)), trace=TRACE
    )
    _CACHED["last_exec_ns"] = res.exec_time_ns
    _CACHED["last_res"] = res

    out = np.zeros((B, N), np.float32)
    for c in range(NCORES):
        out[:, c * NP:(c + 1) * NP] = res.results[c]["out"]

    inv_min = _host_invalid_min(points, tri_verts, valid)
    if inv_min is not None:
        out = np.maximum(out, np.exp(-ALPHA * inv_min).astype(np.float32))
    return out



# revision 16
# speedup vs baseline: 1.3955x; 1.0451x over previous
"""Trainium2 Bass kernel for DifferentiableRasterizer (point-to-mesh distance field).

out[b, n] = exp(-100 * min_f dist^2(points[b,n], tri[b,f]))

Strategy (8-core data-parallel, points axis sharded; tri_verts replicated):
  Host precomputes, per (batch, face), an orthonormal per-segment frame so the
  point-triangle distance decomposes into squares of AFFINE functionals of p:
     dist^2(p, seg_i) = ip_i(p)^2 + delta_i^2,  delta = max(sig-l, min(sig, 0))
     plane^2          = dnp(p)^2
     inside          <=> max_i ip_i(p) <= 0   (ip oriented outward)
     dist^2(p, tri)   = dnp^2 + (inside ? 0 : min_i dist2d_i)
  The 7 affine functionals per face (sig01,sig02,sig12, ip01,ip02,ip12, dnp)
  are computed on the TensorEngine as K=4 matmuls (homogeneous points), and the
  nonlinear tail runs on ACT/DVE/GPSIMD in a face-major layout
  (128 faces on partitions, points along the free dim).
"""

import numpy as np

B = 4
N = 8192
F = 1024
NCORES = 8
NP = N // NCORES          # points per core (per batch)
PC = 512                  # point-chunk (free dim)
NPC = NP // PC            # point chunks per batch
NFC = F // 128            # face chunks per batch
ALPHA = 100.0
MIN_TRI_AREA = 1e-5
BIGVAL = 1e18

# PE matmul dtype mode:
#   "fp32"   exact, 4 cyc/row
#   "fp32r"  1 cyc/row @ N>=256, ~tf32 precision (~5e-3 out err)
#   "bf16x4" 1 cyc/row, K=16 two-limb bf16 split per operand (~1e-4 out err)
MM_MODE = "bf16x4"
KDIM = 16 if MM_MODE == "bf16x4" else 4
USE_CUSTOM = True  # fused custom-DVE ops (7 DVE passes/chunk) vs stock ops
TRACE = False  # set True (before first kernel() call) to capture an NTFF profile

BIGFILL = 1e30   # "not a candidate" fill for inside-masked segment distances
BIGTH = 1e29     # threshold detecting the fill


_DVE_OPS = {}


def _register_custom_ops():
    """Register the two fused DVE ops (idempotent)."""
    if _DVE_OPS:
        return _DVE_OPS
    from concourse.dve_spec import (
        Spec, Src0, Src1, C0, C2, Zero, lower, maxx, minn, select, sq,
    )
    from concourse.dve_ops import DveOp, OPS, get_dve_sub_opcode, has_src1
    from concourse.dve_uop import DveOpSpec
    import numpy as _np

    import concourse.dve_ops as dve_ops_mod

    def _mk(name, spec):
        for op in OPS:
            if op.name == name:
                _DVE_OPS[name] = op
                return
        shas = {}
        op = DveOp(name, spec, subdim=False, uops_sha=shas)
        OPS.append(op)
        # the module builds these maps at import; extend them for new ops
        dve_ops_mod._SUB_OPCODE_FOR_NAME[name] = (
            dve_ops_mod._CUSTOM_DVE_ROW_BASE + len(OPS) - 1
        )
        dve_ops_mod.CUSTOM_DVE_SPECS[name] = spec
        for ver in ("v3", "v4"):
            s = DveOpSpec(
                name=name,
                opcode=get_dve_sub_opcode(name),
                uops=lower(spec, ver=ver),
                rd1_en=has_src1(spec),
            )
            shas[ver] = s.sha(ver)
        _DVE_OPS[name] = op

    # B = select(ip > 0, ip^2 + max(sig + s0, min(sig, 0))^2, BIGFILL)
    #   in0 = sig, in1 = ip, s0 = NEGATED segment length (per-partition),
    #   imm2 = BIGFILL
    _mk(
        "RAST_SEGSEL_ANT",
        Spec(
            body=select(
                Src1 > Zero,
                sq(Src1) + sq(maxx(Src0 + C0, minn(Src0, Zero))),
                C2,
            ),
            reference=lambda in0, in1, s0, imm2: _np.where(
                in1 > 0,
                in1 * in1
                + _np.square(_np.maximum(in0 + s0, _np.minimum(in0, 0.0))),
                imm2,
            ),
        ),
    )
    # fin = dnp^2 + (M >= BIGTH ? 0 : M);  in0 = M, in1 = dnp, s0 = BIGTH
    _mk(
        "RAST_FINPL_ANT",
        Spec(
            body=sq(Src1) + select(Src0 >= C0, Zero, Src0),
            reference=lambda in0, in1, s0: in1 * in1
            + _np.where(in0 >= s0, 0.0, in0),
        ),
    )
    return _DVE_OPS


def _host_face_constants(tri):
    """tri: (B, F, 3, 3) float32 -> per-face affine functional rows (float64)."""
    t = tri.astype(np.float64)
    v0, v1, v2 = t[:, :, 0, :], t[:, :, 1, :], t[:, :, 2, :]
    e0 = v1 - v0
    e1 = v2 - v0
    e12 = v2 - v1
    n = np.cross(e0, e1)
    area2 = (n * n).sum(-1)
    valid = area2 >= 4.0 * (MIN_TRI_AREA ** 2)
    nh = n / np.sqrt(np.maximum(area2, 1e-300))[..., None]

    def seg_const(a, d, opp):
        L = np.sqrt((d * d).sum(-1))
        eh = d / np.maximum(L, 1e-300)[..., None]
        m = np.cross(eh, nh)
        flip = (m * (opp - a)).sum(-1) > 0
        m = np.where(flip[..., None], -m, m)
        # sigma(p) = eh.p + eo ; ip(p) = m.p + mo
        return eh, -(eh * a).sum(-1), m, -(m * a).sum(-1), L

    segs = [seg_const(v0, e0, v2), seg_const(v0, e1, v1), seg_const(v1, e12, v0)]
    dn_c, dn_o = nh, -(nh * v0).sum(-1)

    inv = ~valid
    fixed = []
    for eh, eo, m, mo, L in segs:
        eh = np.where(inv[..., None], 0.0, eh)
        eo = np.where(inv, 0.0, eo)
        m = np.where(inv[..., None], 0.0, m)
        mo = np.where(inv, BIGVAL, mo)
        L = np.where(inv, 1.0, L)
        fixed.append((eh, eo, m, mo, L))
    dn_c = np.where(inv[..., None], 0.0, dn_c)
    dn_o = np.where(inv, BIGVAL, dn_o)
    return fixed, dn_c, dn_o, valid


def _host_pack(points, tri):
    """Build the DRAM input arrays for the device kernel."""
    segs, dn_c, dn_o, valid = _host_face_constants(tri)

    # G matrix: [B, NFC, 7, 4, 128]  (functional rows over homogeneous p)
    # functional order: sig01, sig02, sig12, ip01, ip02, ip12, dnp
    G = np.zeros((B, NFC, 7, 4, 128), np.float32)
    for k in range(3):
        eh, eo, m, mo, _ = segs[k]
        for b in range(B):
            gc = eh[b].reshape(NFC, 128, 3)
            go = eo[b].reshape(NFC, 128)
            G[b, :, k, 0:3, :] = gc.transpose(0, 2, 1)
            G[b, :, k, 3, :] = go
            ic = m[b].reshape(NFC, 128, 3)
            io = mo[b].reshape(NFC, 128)
            G[b, :, 3 + k, 0:3, :] = ic.transpose(0, 2, 1)
            G[b, :, 3 + k, 3, :] = io
    for b in range(B):
        nc_ = dn_c[b].reshape(NFC, 128, 3)
        no_ = dn_o[b].reshape(NFC, 128)
        G[b, :, 6, 0:3, :] = nc_.transpose(0, 2, 1)
        G[b, :, 6, 3, :] = no_

    # negated segment lengths for ACT bias: [128, B*NFC*3]
    lneg = np.zeros((128, B * NFC * 3), np.float32)
    for b in range(B):
        for k in range(3):
            L = segs[k][4][b].reshape(NFC, 128)
            for fc in range(NFC):
                lneg[:, (b * NFC + fc) * 3 + k] = -L[fc]

    eye = np.eye(128, dtype=np.float16)

    # homogeneous point tiles per core: [B, 4, NP] fp32
    pts_full = []
    for c in range(NCORES):
        ps = points[:, c * NP:(c + 1) * NP, :].astype(np.float32)  # (B, NP, 3)
        pt = np.ones((B, 4, NP), np.float32)
        pt[:, 0:3, :] = ps.transpose(0, 2, 1)
        pts_full.append(pt)

    if MM_MODE == "bf16x4":
        import ml_dtypes

        bf16 = ml_dtypes.bfloat16
        # two-limb bf16 split: x = hi + lo (+O(2^-18))
        Ghi = G.astype(bf16)
        Glo = (G - Ghi.astype(np.float32)).astype(bf16)
        # lhsT rows (K=16): [Ghi; Glo; Ghi; Glo]
        Gk = np.concatenate([Ghi, Glo, Ghi, Glo], axis=3)  # [B,NFC,7,16,128]
        Gk = np.ascontiguousarray(Gk.transpose(0, 3, 1, 2, 4)).reshape(
            B, KDIM, NFC * 7 * 128
        )
        ptils = []
        for pt in pts_full:
            phi = pt.astype(bf16)
            plo = (pt - phi.astype(np.float32)).astype(bf16)
            # rhs rows (K=16): [phi; phi; plo; plo]
            ptils.append(np.concatenate([phi, phi, plo, plo], axis=1))
        return Gk, lneg, eye, ptils, valid

    # fp32/fp32r: K=4, pre-transpose for direct DMA (K on partitions)
    Gk = np.ascontiguousarray(G.transpose(0, 3, 1, 2, 4)).reshape(
        B, 4, NFC * 7 * 128
    )
    return Gk, lneg, eye, pts_full, valid


def _host_invalid_min(points, tri, valid):
    """Exact min dist^2 over INVALID faces only (numpy, usually none)."""
    if valid.all():
        return None
    out = np.full((B, N), np.inf, np.float64)
    for b in range(B):
        idx = np.where(~valid[b])[0]
        if len(idx) == 0:
            continue
        t = tri[b, idx].astype(np.float64)   # (Fi, 3, 3)
        p = points[b].astype(np.float64)     # (N, 3)
        v0, v1, v2 = t[:, 0], t[:, 1], t[:, 2]

        def segd(a, d):
            L2 = np.maximum((d * d).sum(-1), 1e-12)
            tt = np.clip(((p[:, None, :] - a) * d).sum(-1) / L2, 0, 1)
            proj = a + tt[..., None] * d
            df = p[:, None, :] - proj
            return (df * df).sum(-1)

        dd = np.minimum(np.minimum(segd(v0, v1 - v0), segd(v0, v2 - v0)),
                        segd(v1, v2 - v1))
        out[b] = dd.min(-1)
    return out


def _build_bass(reps=1):
    import concourse.bass as bass
    import concourse.bacc as bacc
    import concourse.tile as tile
    from concourse import mybir

    f32 = mybir.dt.float32
    nc = bacc.Bacc(None)

    mmdt_in = {
        "fp32": f32,
        "fp32r": mybir.dt.float32r,
        "bf16x4": mybir.dt.bfloat16,
    }[MM_MODE]
    ptil = nc.declare_dram_parameter("ptil", [B, KDIM, NP], mmdt_in, isOutput=False)
    gmat = nc.declare_dram_parameter("gmat", [B, KDIM, NFC * 7 * 128], mmdt_in, isOutput=False)
    lneg = nc.declare_dram_parameter("lneg", [128, B * NFC * 3], f32, isOutput=False)
    eye = nc.declare_dram_parameter("eye", [128, 128], mybir.dt.float16, isOutput=False)
    # [B, partition, NPC*nj] — contiguous per-partition rows so the output
    # DMA needs 128 x 16B descriptors instead of 512 x 4B; host re-permutes
    outp = nc.declare_dram_parameter("out", [B, 128, NP // 128], f32, isOutput=True)

    mm_dt = f32 if MM_MODE == "fp32" else mybir.dt.float32r

    ACT = mybir.ActivationFunctionType
    ALU = mybir.AluOpType
    f16 = mybir.dt.float16

    with tile.TileContext(nc) as tc:
        with (
            tc.tile_pool(name="const", bufs=1) as constp,
            tc.tile_pool(name="gp", bufs=2) as gpool,
            tc.tile_pool(name="pp", bufs=2) as ppool,
            tc.tile_pool(name="ps", bufs=1, space="PSUM") as psum,
            tc.tile_pool(name="pst", bufs=1, space="PSUM") as psum_t,
            tc.tile_pool(name="wk", bufs=2) as wk,
            tc.tile_pool(name="accp", bufs=2) as accp,
            tc.tile_pool(name="outs", bufs=2) as outsp,
        ):
            ltile = constp.tile([128, B * NFC * 3], f32, tag="lneg")
            nc.sync.dma_start(ltile[:], lneg[:])
            eyet = constp.tile([128, 128], f16, tag="eye")
            nc.sync.dma_start(eyet[:], eye[:])

            ops = _register_custom_ops()

            for rep in range(reps):
              for b in range(B):
                # chunk the G-matrix DMA per face-chunk so the first matmul
                # only waits ~1/8th of the transfer (head-latency win)
                gts = []
                for fc in range(NFC):
                    gtc = gpool.tile([KDIM, 7 * 128], mmdt_in, tag=f"g{fc}")
                    nc.sync.dma_start(
                        gtc[:], gmat[b][:, fc * 7 * 128:(fc + 1) * 7 * 128])
                    gts.append(gtc)
                pt = ppool.tile([KDIM, NP], mmdt_in, tag="p")
                nc.sync.dma_start(pt[:], ptil[b])

                for pc in range(NPC):
                    rhs = pt[:, pc * PC:(pc + 1) * PC]
                    acc = accp.tile([128, PC], f16, tag="acc")

                    for fc in range(NFC):
                        # --- PE: 7 affine functionals -> PSUM ---
                        # ip's first (ACT copies drain those banks early),
                        # then sigs, dnp last (consumed last by finpl).
                        fn = {}
                        for phi, name in (
                            (3, "i01"), (4, "i02"), (5, "i12"),
                            (0, "s01"), (1, "s02"), (2, "s12"), (6, "dnp"),
                        ):
                            pst = psum.tile([128, PC], f32, tag=f"ps_{name}")
                            lhsT = gts[fc][:, phi * 128:(phi + 1) * 128]
                            nc.tensor.matmul(pst[:], lhsT, rhs, start=True, stop=True)
                            fn[name] = pst

                        # ip columns to SBUF via ACT (frees a PSUM read for DVE)
                        ics = []
                        for k, inm in enumerate(("i01", "i02", "i12")):
                            ic = wk.tile([128, PC], f32, tag=f"ic_{k}")
                            nc.scalar.activation(ic[:], fn[inm][:], ACT.Copy)
                            ics.append(ic)
                        Bt = []
                        for k, sn in enumerate(("s01", "s02", "s12")):
                            lb = ltile[:, (b * NFC + fc) * 3 + k:
                                       (b * NFC + fc) * 3 + k + 1]
                            Bk = wk.tile([128, PC], f16, tag=f"B_{k}")
                            nc.vector._custom_dve(
                                ops["RAST_SEGSEL_ANT"],
                                out=Bk[:], in0=fn[sn][:], in1=ics[k][:],
                                s0=lb, imm2=BIGFILL,
                            )
                            Bt.append(Bk)
                            if k == 1:
                                m1 = wk.tile([128, PC], f16, tag="m1")
                                nc.vector.tensor_tensor(
                                    m1[:], Bt[0][:], Bt[1][:], op=ALU.min)
                        m2 = wk.tile([128, PC], f16, tag="m2")
                        nc.vector.tensor_tensor(m2[:], m1[:], Bt[2][:], op=ALU.min)
                        if fc == 0:
                            # first face-chunk: finpl writes acc directly
                            # (no memset / no separate accmin needed)
                            nc.vector._custom_dve(
                                ops["RAST_FINPL_ANT"],
                                out=acc[:], in0=m2[:], in1=fn["dnp"][:], s0=BIGTH,
                            )
                        else:
                            fin = wk.tile([128, PC], f16, tag="fin")
                            nc.vector._custom_dve(
                                ops["RAST_FINPL_ANT"],
                                out=fin[:], in0=m2[:], in1=fn["dnp"][:], s0=BIGTH,
                            )
                            nc.vector.tensor_tensor(acc[:], acc[:], fin[:], op=ALU.min)

                    # --- tail: min over the 128 face-slots (partitions) ---
                    nj = PC // 128
                    dmin = outsp.tile([128, nj], f32, tag="dmin")
                    for j in range(nj):
                        tp = psum_t.tile([128, 128], f16, tag="tp")
                        nc.tensor.transpose(tp[:], acc[:, j * 128:(j + 1) * 128], eyet[:])
                        nc.vector.tensor_reduce(
                            dmin[:, j:j + 1], tp[:], axis=mybir.AxisListType.X, op=ALU.min
                        )
                    eo = outsp.tile([128, nj], f32, tag="eo")
                    nc.scalar.activation(eo[:], dmin[:], ACT.Exp, scale=-ALPHA)
                    nc.sync.dma_start(outp[b][:, pc * nj:(pc + 1) * nj], eo[:])

    nc.finalize()
    return nc


_CACHED = {}


def kernel(points: np.ndarray, tri_verts: np.ndarray) -> np.ndarray:
    points = np.asarray(points)
    tri_verts = np.asarray(tri_verts)
    assert points.shape == (B, N, 3) and tri_verts.shape == (B, F, 3, 3)

    from concourse.bass_utils import run_bass_kernel_spmd

    G, lneg, eye, ptils, valid = _host_pack(points, tri_verts)

    if "nc" not in _CACHED:
        _CACHED["nc"] = _build_bass()
    nc = _CACHED["nc"]

    in_maps = [
        {"ptil": ptils[c], "gmat": G, "lneg": lneg, "eye": eye}
        for c in range(NCORES)
    ]
    res = run_bass_kernel_spmd(
        nc, in_maps, core_ids=list(range(NCORES)), trace=TRACE
    )
    _CACHED["last_exec_ns"] = res.exec_time_ns
    _CACHED["last_res"] = res

    out = np.zeros((B, N), np.float32)
    nj = PC // 128
    for c in range(NCORES):
        # device layout [B, p, pc*nj + j] -> point index pc*PC + j*128 + p
        arr = res.results[c]["out"].reshape(B, 128, NPC, nj)
        out[:, c * NP:(c + 1) * NP] = (
            arr.transpose(0, 2, 3, 1).reshape(B, NP)
        )

    inv_min = _host_invalid_min(points, tri_verts, valid)
    if inv_min is not None:
        out = np.maximum(out, np.exp(-ALPHA * inv_min).astype(np.float32))
    return out



# revision 18
# speedup vs baseline: 1.4228x; 1.0196x over previous
"""Trainium2 Bass kernel for DifferentiableRasterizer (point-to-mesh distance field).

out[b, n] = exp(-100 * min_f dist^2(points[b,n], tri[b,f]))

Strategy (8-core data-parallel, points axis sharded; tri_verts replicated):
  Host precomputes, per (batch, face), an orthonormal per-segment frame so the
  point-triangle distance decomposes into squares of AFFINE functionals of p:
     dist^2(p, seg_i) = ip_i(p)^2 + delta_i^2,  delta = max(sig-l, min(sig, 0))
     plane^2          = dnp(p)^2
     inside          <=> max_i ip_i(p) <= 0   (ip oriented outward)
     dist^2(p, tri)   = dnp^2 + (inside ? 0 : min_i dist2d_i)
  The 7 affine functionals per face (sig01,sig02,sig12, ip01,ip02,ip12, dnp)
  are computed on the TensorEngine as K=4 matmuls (homogeneous points), and the
  nonlinear tail runs on ACT/DVE/GPSIMD in a face-major layout
  (128 faces on partitions, points along the free dim).
"""

import numpy as np

B = 4
N = 8192
F = 1024
NCORES = 8
NP = N // NCORES          # points per core (per batch)
PC = 512                  # point-chunk (free dim)
NPC = NP // PC            # point chunks per batch
NFC = F // 128            # face chunks per batch
ALPHA = 100.0
MIN_TRI_AREA = 1e-5
BIGVAL = 1e18

# PE matmul dtype mode:
#   "fp32"   exact, 4 cyc/row
#   "fp32r"  1 cyc/row @ N>=256, ~tf32 precision (~5e-3 out err)
#   "bf16x4" 1 cyc/row, K=16 two-limb bf16 split per operand (~1e-4 out err)
MM_MODE = "bf16x4"
KDIM = 16 if MM_MODE == "bf16x4" else 4
USE_CUSTOM = True  # fused custom-DVE ops (7 DVE passes/chunk) vs stock ops
TRACE = False  # set True (before first kernel() call) to capture an NTFF profile

BIGFILL = 1e30   # "not a candidate" fill for inside-masked segment distances
BIGTH = 1e29     # threshold detecting the fill


_DVE_OPS = {}


def _register_custom_ops():
    """Register the two fused DVE ops (idempotent)."""
    if _DVE_OPS:
        return _DVE_OPS
    from concourse.dve_spec import (
        Spec, Src0, Src1, C0, C2, Zero, lower, maxx, minn, select, sq,
    )
    from concourse.dve_ops import DveOp, OPS, get_dve_sub_opcode, has_src1
    from concourse.dve_uop import DveOpSpec
    import numpy as _np

    import concourse.dve_ops as dve_ops_mod

    def _mk(name, spec):
        for op in OPS:
            if op.name == name:
                _DVE_OPS[name] = op
                return
        shas = {}
        op = DveOp(name, spec, subdim=False, uops_sha=shas)
        OPS.append(op)
        # the module builds these maps at import; extend them for new ops
        dve_ops_mod._SUB_OPCODE_FOR_NAME[name] = (
            dve_ops_mod._CUSTOM_DVE_ROW_BASE + len(OPS) - 1
        )
        dve_ops_mod.CUSTOM_DVE_SPECS[name] = spec
        for ver in ("v3", "v4"):
            s = DveOpSpec(
                name=name,
                opcode=get_dve_sub_opcode(name),
                uops=lower(spec, ver=ver),
                rd1_en=has_src1(spec),
            )
            shas[ver] = s.sha(ver)
        _DVE_OPS[name] = op

    # B = select(ip > 0, ip^2 + max(sig + s0, min(sig, 0))^2, BIGFILL)
    #   in0 = sig, in1 = ip, s0 = NEGATED segment length (per-partition),
    #   imm2 = BIGFILL
    _mk(
        "RAST_SEGSEL_ANT",
        Spec(
            body=select(
                Src1 > Zero,
                sq(Src1) + sq(maxx(Src0 + C0, minn(Src0, Zero))),
                C2,
            ),
            reference=lambda in0, in1, s0, imm2: _np.where(
                in1 > 0,
                in1 * in1
                + _np.square(_np.maximum(in0 + s0, _np.minimum(in0, 0.0))),
                imm2,
            ),
        ),
    )
    # fin = dnp^2 + (M >= BIGTH ? 0 : M);  in0 = M, in1 = dnp, s0 = BIGTH
    _mk(
        "RAST_FINPL_ANT",
        Spec(
            body=sq(Src1) + select(Src0 >= C0, Zero, Src0),
            reference=lambda in0, in1, s0: in1 * in1
            + _np.where(in0 >= s0, 0.0, in0),
        ),
    )
    return _DVE_OPS


def _host_face_constants(tri):
    """tri: (B, F, 3, 3) float32 -> per-face affine functional rows (float64)."""
    t = tri.astype(np.float64)
    v0, v1, v2 = t[:, :, 0, :], t[:, :, 1, :], t[:, :, 2, :]
    e0 = v1 - v0
    e1 = v2 - v0
    e12 = v2 - v1
    n = np.cross(e0, e1)
    area2 = (n * n).sum(-1)
    valid = area2 >= 4.0 * (MIN_TRI_AREA ** 2)
    nh = n / np.sqrt(np.maximum(area2, 1e-300))[..., None]

    def seg_const(a, d, opp):
        L = np.sqrt((d * d).sum(-1))
        eh = d / np.maximum(L, 1e-300)[..., None]
        m = np.cross(eh, nh)
        flip = (m * (opp - a)).sum(-1) > 0
        m = np.where(flip[..., None], -m, m)
        # sigma(p) = eh.p + eo ; ip(p) = m.p + mo
        return eh, -(eh * a).sum(-1), m, -(m * a).sum(-1), L

    segs = [seg_const(v0, e0, v2), seg_const(v0, e1, v1), seg_const(v1, e12, v0)]
    dn_c, dn_o = nh, -(nh * v0).sum(-1)

    inv = ~valid
    fixed = []
    for eh, eo, m, mo, L in segs:
        eh = np.where(inv[..., None], 0.0, eh)
        eo = np.where(inv, 0.0, eo)
        m = np.where(inv[..., None], 0.0, m)
        mo = np.where(inv, BIGVAL, mo)
        L = np.where(inv, 1.0, L)
        fixed.append((eh, eo, m, mo, L))
    dn_c = np.where(inv[..., None], 0.0, dn_c)
    dn_o = np.where(inv, BIGVAL, dn_o)
    return fixed, dn_c, dn_o, valid


def _host_pack(points, tri):
    """Build the DRAM input arrays for the device kernel."""
    segs, dn_c, dn_o, valid = _host_face_constants(tri)

    # G matrix: [B, NFC, 7, 4, 128]  (functional rows over homogeneous p)
    # functional order: sig01, sig02, sig12, ip01, ip02, ip12, dnp
    G = np.zeros((B, NFC, 7, 4, 128), np.float32)
    for k in range(3):
        eh, eo, m, mo, _ = segs[k]
        for b in range(B):
            gc = eh[b].reshape(NFC, 128, 3)
            go = eo[b].reshape(NFC, 128)
            G[b, :, k, 0:3, :] = gc.transpose(0, 2, 1)
            G[b, :, k, 3, :] = go
            ic = m[b].reshape(NFC, 128, 3)
            io = mo[b].reshape(NFC, 128)
            G[b, :, 3 + k, 0:3, :] = ic.transpose(0, 2, 1)
            G[b, :, 3 + k, 3, :] = io
    for b in range(B):
        nc_ = dn_c[b].reshape(NFC, 128, 3)
        no_ = dn_o[b].reshape(NFC, 128)
        G[b, :, 6, 0:3, :] = nc_.transpose(0, 2, 1)
        G[b, :, 6, 3, :] = no_

    # negated segment lengths for ACT bias: [128, B*NFC*3]
    lneg = np.zeros((128, B * NFC * 3), np.float32)
    for b in range(B):
        for k in range(3):
            L = segs[k][4][b].reshape(NFC, 128)
            for fc in range(NFC):
                lneg[:, (b * NFC + fc) * 3 + k] = -L[fc]

    eye = np.eye(128, dtype=np.float16)

    # homogeneous point tiles per core: [B, 4, NP] fp32
    pts_full = []
    for c in range(NCORES):
        ps = points[:, c * NP:(c + 1) * NP, :].astype(np.float32)  # (B, NP, 3)
        pt = np.ones((B, 4, NP), np.float32)
        pt[:, 0:3, :] = ps.transpose(0, 2, 1)
        pts_full.append(pt)

    if MM_MODE == "bf16x4":
        import ml_dtypes

        bf16 = ml_dtypes.bfloat16
        # two-limb bf16 split: x = hi + lo (+O(2^-18))
        Ghi = G.astype(bf16)
        Glo = (G - Ghi.astype(np.float32)).astype(bf16)
        # lhsT rows (K=16): [Ghi; Glo; Ghi; Glo]
        Gk = np.concatenate([Ghi, Glo, Ghi, Glo], axis=3)  # [B,NFC,7,16,128]
        Gk = np.ascontiguousarray(Gk.transpose(0, 3, 1, 2, 4)).reshape(
            B, KDIM, NFC * 7 * 128
        )
        ptils = []
        for pt in pts_full:
            phi = pt.astype(bf16)
            plo = (pt - phi.astype(np.float32)).astype(bf16)
            # rhs rows (K=16): [phi; phi; plo; plo]
            ptils.append(np.concatenate([phi, phi, plo, plo], axis=1))
        return Gk, lneg, eye, ptils, valid

    # fp32/fp32r: K=4, pre-transpose for direct DMA (K on partitions)
    Gk = np.ascontiguousarray(G.transpose(0, 3, 1, 2, 4)).reshape(
        B, 4, NFC * 7 * 128
    )
    return Gk, lneg, eye, pts_full, valid


def _host_invalid_min(points, tri, valid):
    """Exact min dist^2 over INVALID faces only (numpy, usually none)."""
    if valid.all():
        return None
    out = np.full((B, N), np.inf, np.float64)
    for b in range(B):
        idx = np.where(~valid[b])[0]
        if len(idx) == 0:
            continue
        t = tri[b, idx].astype(np.float64)   # (Fi, 3, 3)
        p = points[b].astype(np.float64)     # (N, 3)
        v0, v1, v2 = t[:, 0], t[:, 1], t[:, 2]

        def segd(a, d):
            L2 = np.maximum((d * d).sum(-1), 1e-12)
            tt = np.clip(((p[:, None, :] - a) * d).sum(-1) / L2, 0, 1)
            proj = a + tt[..., None] * d
            df = p[:, None, :] - proj
            return (df * df).sum(-1)

        dd = np.minimum(np.minimum(segd(v0, v1 - v0), segd(v0, v2 - v0)),
                        segd(v1, v2 - v1))
        out[b] = dd.min(-1)
    return out


def _build_bass(reps=1):
    import concourse.bass as bass
    import concourse.bacc as bacc
    import concourse.tile as tile
    from concourse import mybir

    f32 = mybir.dt.float32
    nc = bacc.Bacc(None)

    mmdt_in = {
        "fp32": f32,
        "fp32r": mybir.dt.float32r,
        "bf16x4": mybir.dt.bfloat16,
    }[MM_MODE]
    ptil = nc.declare_dram_parameter("ptil", [B, KDIM, NP], mmdt_in, isOutput=False)
    gmat = nc.declare_dram_parameter("gmat", [B, KDIM, NFC * 7 * 128], mmdt_in, isOutput=False)
    lneg = nc.declare_dram_parameter("lneg", [128, B * NFC * 3], f32, isOutput=False)
    eye = nc.declare_dram_parameter("eye", [128, 128], mybir.dt.float16, isOutput=False)
    # [B, partition, NPC*nj] — contiguous per-partition rows so the output
    # DMA needs 128 x 16B descriptors instead of 512 x 4B; host re-permutes
    outp = nc.declare_dram_parameter("out", [B, 128, NP // 128], f32, isOutput=True)

    mm_dt = f32 if MM_MODE == "fp32" else mybir.dt.float32r

    ACT = mybir.ActivationFunctionType
    ALU = mybir.AluOpType
    f16 = mybir.dt.float16

    with tile.TileContext(nc) as tc:
        with (
            tc.tile_pool(name="const", bufs=1) as constp,
            tc.tile_pool(name="gp", bufs=2) as gpool,
            tc.tile_pool(name="pp", bufs=2) as ppool,
            tc.tile_pool(name="ps", bufs=1, space="PSUM") as psum,
            tc.tile_pool(name="pst", bufs=1, space="PSUM") as psum_t,
            tc.tile_pool(name="wk", bufs=3) as wk,
            tc.tile_pool(name="accp", bufs=2) as accp,
            tc.tile_pool(name="outs", bufs=2) as outsp,
        ):
            ops = _register_custom_ops()
            ltile = eyet = None

            for rep in range(reps):
              for b in range(B):
                # points first (every matmul needs them), then the G chunks
                # (first matmul only waits ~1/8th of the transfer), then the
                # later-consumed constants on the first pass
                pt = ppool.tile([KDIM, NP], mmdt_in, tag="p")
                nc.sync.dma_start(pt[:], ptil[b])
                gts = []
                for fc in range(NFC):
                    gtc = gpool.tile([KDIM, 7 * 128], mmdt_in, tag=f"g{fc}")
                    nc.sync.dma_start(
                        gtc[:], gmat[b][:, fc * 7 * 128:(fc + 1) * 7 * 128])
                    gts.append(gtc)
                    if fc == 0 and ltile is None:
                        ltile = constp.tile([128, B * NFC * 3], f32, tag="lneg")
                        nc.sync.dma_start(ltile[:], lneg[:])
                        eyet = constp.tile([128, 128], f16, tag="eye")
                        nc.sync.dma_start(eyet[:], eye[:])

                for pc in range(NPC):
                    rhs = pt[:, pc * PC:(pc + 1) * PC]
                    acc = accp.tile([128, PC], f16, tag="acc")

                    for fc in range(NFC):
                        # --- PE: 7 affine functionals -> PSUM ---
                        # ip's first (ACT copies drain those banks early),
                        # then sigs, dnp last (consumed last by finpl).
                        fn = {}
                        for phi, name in (
                            (3, "i01"), (4, "i02"), (5, "i12"),
                            (0, "s01"), (1, "s02"), (2, "s12"), (6, "dnp"),
                        ):
                            pst = psum.tile([128, PC], f32, tag=f"ps_{name}")
                            lhsT = gts[fc][:, phi * 128:(phi + 1) * 128]
                            nc.tensor.matmul(pst[:], lhsT, rhs, start=True, stop=True)
                            fn[name] = pst

                        # ip columns to SBUF via ACT (frees a PSUM read for DVE)
                        ics = []
                        for k, inm in enumerate(("i01", "i02", "i12")):
                            ic = wk.tile([128, PC], f32, tag=f"ic_{k}")
                            nc.scalar.activation(ic[:], fn[inm][:], ACT.Copy)
                            ics.append(ic)
                        Bt = []
                        for k, sn in enumerate(("s01", "s02", "s12")):
                            lb = ltile[:, (b * NFC + fc) * 3 + k:
                                       (b * NFC + fc) * 3 + k + 1]
                            Bk = wk.tile([128, PC], f16, tag=f"B_{k}")
                            nc.vector._custom_dve(
                                ops["RAST_SEGSEL_ANT"],
                                out=Bk[:], in0=fn[sn][:], in1=ics[k][:],
                                s0=lb, imm2=BIGFILL,
                            )
                            Bt.append(Bk)
                            if k == 1:
                                m1 = wk.tile([128, PC], f16, tag="m1")
                                nc.vector.tensor_tensor(
                                    m1[:], Bt[0][:], Bt[1][:], op=ALU.min)
                        m2 = wk.tile([128, PC], f16, tag="m2")
                        nc.vector.tensor_tensor(m2[:], m1[:], Bt[2][:], op=ALU.min)
                        if fc == 0:
                            # first face-chunk: finpl writes acc directly
                            # (no memset / no separate accmin needed)
                            nc.vector._custom_dve(
                                ops["RAST_FINPL_ANT"],
                                out=acc[:], in0=m2[:], in1=fn["dnp"][:], s0=BIGTH,
                            )
                        else:
                            fin = wk.tile([128, PC], f16, tag="fin")
                            nc.vector._custom_dve(
                                ops["RAST_FINPL_ANT"],
                                out=fin[:], in0=m2[:], in1=fn["dnp"][:], s0=BIGTH,
                            )
                            nc.vector.tensor_tensor(acc[:], acc[:], fin[:], op=ALU.min)

                    # --- tail: min over the 128 face-slots (partitions) ---
                    nj = PC // 128
                    dmin = outsp.tile([128, nj], f32, tag="dmin")
                    for j in range(nj):
                        tp = psum_t.tile([128, 128], f16, tag="tp")
                        nc.tensor.transpose(tp[:], acc[:, j * 128:(j + 1) * 128], eyet[:])
                        nc.vector.tensor_reduce(
                            dmin[:, j:j + 1], tp[:], axis=mybir.AxisListType.X, op=ALU.min
                        )
                    eo = outsp.tile([128, nj], f32, tag="eo")
                    nc.scalar.activation(eo[:], dmin[:], ACT.Exp, scale=-ALPHA)
                    nc.sync.dma_start(outp[b][:, pc * nj:(pc + 1) * nj], eo[:])

    nc.finalize()
    return nc


_CACHED = {}


def kernel(points: np.ndarray, tri_verts: np.ndarray) -> np.ndarray:
    points = np.asarray(points)
    tri_verts = np.asarray(tri_verts)
    assert points.shape == (B, N, 3) and tri_verts.shape == (B, F, 3, 3)

    from concourse.bass_utils import run_bass_kernel_spmd

    G, lneg, eye, ptils, valid = _host_pack(points, tri_verts)

    if "nc" not in _CACHED:
        _CACHED["nc"] = _build_bass()
    nc = _CACHED["nc"]

    in_maps = [
        {"ptil": ptils[c], "gmat": G, "lneg": lneg, "eye": eye}
        for c in range(NCORES)
    ]
    res = run_bass_kernel_spmd(
        nc, in_maps, core_ids=list(range(NCORES)), trace=TRACE
    )
    _CACHED["last_exec_ns"] = res.exec_time_ns
    _CACHED["last_res"] = res

    out = np.zeros((B, N), np.float32)
    nj = PC // 128
    for c in range(NCORES):
        # device layout [B, p, pc*nj + j] -> point index pc*PC + j*128 + p
        arr = res.results[c]["out"].reshape(B, 128, NPC, nj)
        out[:, c * NP:(c + 1) * NP] = (
            arr.transpose(0, 2, 3, 1).reshape(B, NP)
        )

    inv_min = _host_invalid_min(points, tri_verts, valid)
    if inv_min is not None:
        out = np.maximum(out, np.exp(-ALPHA * inv_min).astype(np.float32))
    return out

